# revision 6
# baseline (speedup 1.0000x reference)
"""Trainium2 Bass kernel for a custom GRU (nn_BasicGRU).

Reference computation (per batch row b, h0 = 0):
    for t in 0..T-1:
        comb  = [x_t, h]                          # [I+H]
        z     = sigmoid(comb @ Wz + bz)
        r     = sigmoid(comb @ Wr + br)
        comb2 = [x_t, r*h]
        hc    = tanh(comb2 @ Wh + bh)
        h     = (1-z)*h + z*hc
        y_t   = h

Shapes: x [128, 1024, 256] f32, W* [768, 512] f32, b* [512] f32,
y [128, 1024, 512] f32.

Strategy (8 NeuronCores, data-parallel over batch, 16 rows/core):
- All on-chip state kept "H-major": feature dim on partitions, batch in the
  free dim.  State h is [128 partitions, 4 chunks, 16 batch] (feature
  f = chunk*128 + partition).  This makes every per-step elementwise op a
  cheap [128, 64] op and avoids any transposes in the recurrence.
- Weights are the matmul stationary operand (lhsT = W[kchunk, mchunk] tile),
  the streamed operand is the small h / (r*h) tile [128, 16].
- The x-dependent part of all three gate pre-activations (x_t @ Wx_g + b_g)
  does not depend on the recurrence: it is computed by efficient batched
  matmuls directly into the same PSUM banks the recurrent matmuls then
  accumulate into (one bank per gate per 8-step block).
- Recurrent weights and gate weights are cast to bf16 (fp32 PSUM
  accumulation); the master h state stays fp32, with a bf16 copy made each
  step for the next step's matmuls.
- Output y is written H-major to HBM and rearranged to [B, T, H] on host.
"""

import numpy as np
import ml_dtypes

import concourse.bass as bass
import concourse.tile as tile
from concourse import bacc, mybir
from concourse.bass_utils import run_bass_kernel_spmd

F32 = mybir.dt.float32
BF16 = mybir.dt.bfloat16

N_CORES = 8
B = 128
I_DIM = 256
H_DIM = 512
B_LOC = B // N_CORES          # 16 batch rows per core
BLK = 8                       # recurrence steps per PSUM block
KC = H_DIM // 128             # 4 k-chunks for the h-part contraction
MC = H_DIM // 128             # 4 output-feature chunks
IC = I_DIM // 128             # 2 k-chunks for the x-part contraction
GZ, GR, GH = 0, 1, 2          # gate order in the stacked weight tensors

_CACHE = {}


def build_gru_nc(T):
    """Build the Bass/Tile program for a T-step GRU on one core."""
    NB = T // BLK
    assert T % BLK == 0

    nc = bacc.Bacc("TRN2", target_bir_lowering=False, debug=False,
                   enable_asserts=False, num_devices=N_CORES)

    xT = nc.dram_tensor("xT", [IC, 128, T * B_LOC], BF16, kind="ExternalInput")
    whh = nc.dram_tensor("whh", [128, 3, KC, MC, 128], BF16, kind="ExternalInput")
    wx = nc.dram_tensor("wx", [128, 3, IC, MC, 128], BF16, kind="ExternalInput")
    bias = nc.dram_tensor("bias", [1, 3, MC, 128], BF16, kind="ExternalInput")
    y = nc.dram_tensor("y", [128, MC, T * B_LOC], F32, kind="ExternalOutput")

    with tile.TileContext(nc) as tc:
        with (
            tc.tile_pool(name="const", bufs=1) as const,
            tc.tile_pool(name="xp", bufs=3) as xp,
            tc.tile_pool(name="yp", bufs=2) as yp,
            tc.tile_pool(name="hp", bufs=2) as hp,
            tc.tile_pool(name="sp", bufs=2) as sp,
            tc.tile_pool(name="ps", bufs=2, space="PSUM") as ps,
        ):
            # ---- constants ----
            whh_s = const.tile([128, 3, KC, MC, 128], BF16, tag="whh")
            nc.sync.dma_start(whh_s[:], whh[:])
            wx_s = const.tile([128, 3, IC, MC, 128], BF16, tag="wx")
            nc.sync.dma_start(wx_s[:], wx[:])
            bias_s = const.tile([1, 3, MC, 128], BF16, tag="bias")
            nc.sync.dma_start(bias_s[:], bias[:])
            ones_s = const.tile([1, BLK * B_LOC], BF16, tag="ones")
            nc.vector.memset(ones_s[:], 1.0)
            h0_b = const.tile([128, KC, B_LOC], BF16, tag="h0b")
            nc.vector.memset(h0_b[:], 0.0)
            h0_f = const.tile([128, KC, B_LOC], F32, tag="h0f")
            nc.vector.memset(h0_f[:], 0.0)

            h_prev_b = h0_b[:]
            h_prev_f = h0_f[:]

            def alloc_block(blk):
                """Allocate block tiles, start the x DMA, and return the
                per-block state; x-part matmuls are emitted separately so
                they can interleave with the previous block's steps."""
                c0 = blk * BLK * B_LOC
                c1 = (blk + 1) * BLK * B_LOC
                xt = xp.tile([128, IC, BLK * B_LOC], BF16, tag="xt",
                             name="xt")
                nc.sync.dma_start(xt[:],
                                  xT[:, :, c0:c1].rearrange("i p n -> p i n"))
                psg = [ps.tile([128, MC, BLK * B_LOC], F32, tag=f"ps{g}",
                               name=f"ps{g}") for g in range(3)]
                # x-part matmul work list: 36 (gate, mc, *) matmuls + bias
                work = []
                for g in range(3):
                    for mc in range(MC):
                        work.append((psg[g][:, mc, :], wx_s[:, g, 0, mc, :],
                                     xt[:, 0, :], mc == 0))
                        work.append((psg[g][:, mc, :], wx_s[:, g, 1, mc, :],
                                     xt[:, 1, :], False))
                        work.append((psg[g][:, mc, :], bias_s[:, g, mc, :],
                                     ones_s[:], False))
                return psg, work

            def emit_xpre(work, n):
                for _ in range(min(n, len(work))):
                    out_ap, lhsT, rhs, is_start = work.pop(0)
                    nc.tensor.matmul(out_ap, lhsT, rhs,
                                     start=is_start, stop=False)

            psg, xwork = alloc_block(0)
            emit_xpre(xwork, len(xwork))

            for blk in range(NB):
                c0 = blk * BLK * B_LOC
                c1 = (blk + 1) * BLK * B_LOC
                ys = yp.tile([128, MC, BLK * B_LOC], F32, tag="ys")
                if blk + 1 < NB:
                    psg_next, xwork = alloc_block(blk + 1)
                else:
                    psg_next, xwork = None, []

                for tl in range(BLK):
                    s0 = tl * B_LOC
                    s1 = (tl + 1) * B_LOC
                    # r then z gate: accumulate h @ Whh_g onto the x-part
                    for g in (GR, GZ):
                        for mc in range(MC):
                            for kc in range(KC):
                                nc.tensor.matmul(
                                    psg[g][:, mc, s0:s1],
                                    whh_s[:, g, kc, mc, :],
                                    h_prev_b[:, kc, :],
                                    start=False, stop=(kc == KC - 1),
                                )
                    r_b = sp.tile([128, MC, B_LOC], BF16, tag="r_b")
                    nc.scalar.activation(r_b[:], psg[GR][:, :, s0:s1],
                                         func=mybir.ActivationFunctionType.Sigmoid)
                    rh_b = sp.tile([128, MC, B_LOC], BF16, tag="rh_b")
                    nc.vector.tensor_mul(rh_b[:], r_b[:], h_prev_b)
                    z_b = sp.tile([128, MC, B_LOC], BF16, tag="z_b")
                    nc.scalar.activation(z_b[:], psg[GZ][:, :, s0:s1],
                                         func=mybir.ActivationFunctionType.Sigmoid)
                    # off-path fused: negb = (z - 1) * h   (fp32)
                    negb_f = sp.tile([128, MC, B_LOC], F32, tag="negb_f")
                    nc.vector.scalar_tensor_tensor(
                        negb_f[:], z_b[:], 1.0, h_prev_f,
                        op0=mybir.AluOpType.subtract,
                        op1=mybir.AluOpType.mult)

                    # candidate gate: (r*h) @ Whh_h
                    for mc in range(MC):
                        for kc in range(KC):
                            nc.tensor.matmul(
                                psg[GH][:, mc, s0:s1],
                                whh_s[:, GH, kc, mc, :],
                                rh_b[:, kc, :],
                                start=False, stop=(kc == KC - 1),
                            )
                    # next block's x-part matmuls fill the PE idle gap while
                    # the tanh/blend tail runs
                    emit_xpre(xwork, 5)
                    hc_f = sp.tile([128, MC, B_LOC], F32, tag="hc_f")
                    nc.scalar.activation(hc_f[:], psg[GH][:, :, s0:s1],
                                         func=mybir.ActivationFunctionType.Tanh)

                    # blend h' = z*hc - (z-1)*h:
                    # on-path: a = z*hc, then h'_bf16 = a - negb (bf16 out
                    # feeds the next step's matmuls directly); the fp32
                    # master copy (into y staging) is off the critical path.
                    a_f = sp.tile([128, MC, B_LOC], F32, tag="a_f")
                    nc.vector.tensor_mul(a_f[:], z_b[:], hc_f[:])
                    h_b = hp.tile([128, KC, B_LOC], BF16, tag="h_b")
                    nc.vector.tensor_sub(h_b[:], a_f[:], negb_f[:])
                    h_new_f = ys[:, :, s0:s1]
                    nc.vector.tensor_sub(h_new_f, a_f[:], negb_f[:])

                    h_prev_b = h_b[:]
                    h_prev_f = h_new_f

                emit_xpre(xwork, len(xwork))
                nc.gpsimd.dma_start(y[:, :, c0:c1], ys[:])
                psg = psg_next

    nc.finalize()
    return nc


def _host_prep_weights(Wz, bz, Wr, br, Wh, bh):
    Wst = np.stack([Wz, Wr, Wh])                     # [3, 768, 512]
    wx_host = np.ascontiguousarray(
        Wst[:, :I_DIM, :].reshape(3, IC, 128, MC, 128).transpose(2, 0, 1, 3, 4)
    ).astype(ml_dtypes.bfloat16)                     # [128, 3, IC, MC, 128]
    whh_host = np.ascontiguousarray(
        Wst[:, I_DIM:, :].reshape(3, KC, 128, MC, 128).transpose(2, 0, 1, 3, 4)
    ).astype(ml_dtypes.bfloat16)                     # [128, 3, KC, MC, 128]
    bias_host = np.stack([bz, br, bh]).reshape(1, 3, MC, 128).astype(
        ml_dtypes.bfloat16)
    return wx_host, whh_host, bias_host


def make_in_maps(x, Wz, bz, Wr, br, Wh, bh):
    x = np.asarray(x)
    T = x.shape[1]
    assert x.shape == (B, T, I_DIM)
    wx_host, whh_host, bias_host = _host_prep_weights(
        np.asarray(Wz), np.asarray(bz), np.asarray(Wr), np.asarray(br),
        np.asarray(Wh), np.asarray(bh))
    in_maps = []
    for c in range(N_CORES):
        xc = x[c * B_LOC:(c + 1) * B_LOC]            # [16, T, 256]
        xTc = np.ascontiguousarray(xc.transpose(2, 1, 0)).reshape(
            IC, 128, T * B_LOC).astype(ml_dtypes.bfloat16)
        in_maps.append({
            "xT": xTc,
            "whh": whh_host,
            "wx": wx_host,
            "bias": bias_host,
        })
    return in_maps


def assemble_output(y_cat, T):
    """y_cat: [N_CORES*128, MC, T*B_LOC] (concatenated per-core 'y' outputs)
    -> [B, T, H]."""
    y_cat = np.asarray(y_cat).reshape(N_CORES, 128, MC, T, B_LOC)
    out = y_cat.transpose(0, 4, 3, 2, 1).reshape(B, T, H_DIM)
    return np.ascontiguousarray(out, dtype=np.float32)


def kernel(x, Wz, bz, Wr, br, Wh, bh):
    x = np.asarray(x)
    T = x.shape[1]
    in_maps = make_in_maps(x, Wz, bz, Wr, br, Wh, bh)

    if T not in _CACHE:
        _CACHE[T] = build_gru_nc(T)
    nc = _CACHE[T]

    res = run_bass_kernel_spmd(nc, in_maps, core_ids=list(range(N_CORES)))
    y_cat = np.concatenate([res.results[c]["y"] for c in range(N_CORES)], axis=0)
    return assemble_output(y_cat, T)
